# revision 1
# baseline (speedup 1.0000x reference)
"""Cross-modality attention Trainium2 kernel (8 NeuronCores, SPMD).

Problem: 3 modalities feat0..2 of [B=2, C=256, H=W=48]. For each modality i:
  ctx_i = sum_j softmax(Q_i K_j^T / sqrt(C)) V_j   (9 full NxN attentions, N=2304)
  out_i = Conv1x1(gate_i * ctx_i + (1-gate_i) * feat_i),  gate_i = sigmoid(Conv1x1(feat_i))

Sharding: core r handles batch b = r//4 and query-token slice s = r%4 (576 of the
2304 tokens) for all 3 modalities, flash-attention style with K/V replicated
(computed on-device from the full per-batch features).

Device algorithm per core (all matmuls bf16, fp32 PSUM accumulation):
- Q/K projections are folded into one matrix on the host:
  scores^T = X_j^T (Wk^T Wq / sqrt(C)) X_i = X_j^T Qg_i, so only one projection
  (Qg) per modality and the K projection/storage disappears entirely.
- Scores are computed TRANSPOSED ([key, query] layout) directly by matmul, so no
  transpose of the probability matrix is ever needed.  exp() without
  max-subtraction (scores are bounded: |s| < ~2 for this problem's distribution).
- V^T is computed directly in [token, channel] layout with a ones-column
  appended; the attention row-sum (softmax denominator) then falls out of the
  same matmul that computes P @ V, and normalization is a per-partition
  scalar multiply fused with the cross-modality accumulation.
"""

import os
from contextlib import ExitStack

import ml_dtypes
import numpy as np

import concourse.bass as bass
import concourse.tile as tile
from concourse import bacc, mybir
from concourse.bass_utils import run_bass_kernel_spmd
from concourse.masks import make_identity

B, C, H, W = 2, 256, 48, 48
N = H * W            # 2304 tokens
NCORES = 8
NSLICE = 4           # query slices per batch
QS = N // NSLICE     # 576 query tokens per core
KC = N // 128        # 18 key chunks of 128
QH = QS // 2         # 288: scores free-dim half (one PSUM bank)
MQ = (QS + 127) // 128  # 5 query m-chunks for the ctx matmul (last is 64)
XCH = N // QS        # 4 x-load chunks of 576 per half

F32 = mybir.dt.float32
BF16 = mybir.dt.bfloat16
AF = mybir.ActivationFunctionType
ALU = mybir.AluOpType


def _emit(ctx: ExitStack, tc: tile.TileContext, io: dict):
    nc = tc.nc

    # ---- pools ----------------------------------------------------------
    P = 128
    persist = ctx.enter_context(tc.tile_pool(name="persist", bufs=1))
    stgx = ctx.enter_context(tc.tile_pool(name="stgx", bufs=6))
    stgq = ctx.enter_context(tc.tile_pool(name="stgq", bufs=2))
    pt_pool = ctx.enter_context(tc.tile_pool(name="pt", bufs=2))
    ctx_pool = ctx.enter_context(tc.tile_pool(name="ctxp", bufs=2))
    ctxt_pool = ctx.enter_context(tc.tile_pool(name="ctxt", bufs=2))
    fus_pool = ctx.enter_context(tc.tile_pool(name="fus", bufs=2))
    tmpq_pool = ctx.enter_context(tc.tile_pool(name="tmpq", bufs=2))
    rp_pool = ctx.enter_context(tc.tile_pool(name="rp", bufs=6))
    osb_pool = ctx.enter_context(tc.tile_pool(name="osb", bufs=3))
    ps_s = ctx.enter_context(tc.tile_pool(name="ps_s", bufs=2, space="PSUM"))
    ps_c = ctx.enter_context(tc.tile_pool(name="ps_c", bufs=2, space="PSUM"))
    ps_w = ctx.enter_context(tc.tile_pool(name="ps_w", bufs=2, space="PSUM"))

    # ---- constants / weights (host pre-casts weights to bf16) -----------
    pos_sb = persist.tile([P, 2, 1], F32, tag="pos")
    gb_sb = persist.tile([P, 2, 1], F32, tag="gb")
    vb3_sb = persist.tile([P, 2, 1], F32, tag="vb3")
    outb_sb = persist.tile([P, 2, 1], F32, tag="outb")
    nc.sync.dma_start(out=pos_sb[:, :, 0],
                      in_=io["pos"].rearrange("(t p) -> p t", p=P))

    wnames = ["m_t", "v_wt", "gate_wt", "out_wt"]
    wbf = {}
    for wn in wnames:
        wbf[wn] = persist.tile([P, 2, C], BF16, tag=f"w_{wn}", name=f"w_{wn}")

    def load_weight(wn):
        nc.sync.dma_start(out=wbf[wn][:, :, :],
                          in_=io[wn].rearrange("(t p) d -> p t d", p=P))

    def load_consts():
        for nm, t_ in (("gate_b", gb_sb), ("v_b3", vb3_sb), ("out_b", outb_sb)):
            nc.sync.dma_start(out=t_[:, :, 0],
                              in_=io[nm].rearrange("(t p) -> p t", p=P))

    xbf = [persist.tile([P, 2, N], BF16, tag=f"xbf{j}", name=f"xbf{j}")
           for j in range(3)]
    xqbf = [persist.tile([P, 2, QS], BF16, tag=f"xqbf{i}", name=f"xqbf{i}")
            for i in range(3)]
    vst = [persist.tile([P, KC, 257], BF16, tag=f"vst{j}", name=f"vst{j}")
           for j in range(3)]
    qg = [persist.tile([P, 2, QS], BF16, tag=f"qg{i}", name=f"qg{i}")
          for i in range(3)]
    gate = [persist.tile([P, 2, QS], BF16, tag=f"gate{i}", name=f"gate{i}")
            for i in range(3)]

    def load_xq(i):
        for t in range(2):
            stg = stgq.tile([P, QS], F32, tag="stgq")
            nc.sync.dma_start(out=stg[:, :],
                              in_=io[f"xq{i}"][t * P:(t + 1) * P, :])
            nc.scalar.activation(out=xqbf[i][:, t, :], in_=stg[:, :],
                                 func=AF.Identity, bias=pos_sb[:, t, :],
                                 scale=1.0)

    def load_x(j):
        # chunked [128, 576] loads so downstream compute starts early
        for t in range(2):
            for xc in range(XCH):
                stg = stgx.tile([P, QS], F32, tag="stgx")
                nc.gpsimd.dma_start(
                    out=stg[:, :],
                    in_=io[f"x{j}"][t * P:(t + 1) * P, xc * QS:(xc + 1) * QS])
                nc.vector.tensor_scalar_add(xbf[j][:, t, xc * QS:(xc + 1) * QS],
                                            stg[:, :], pos_sb[:, t, :])

    def project_v(j):
        # V^T[n, c] = sum_cin X[cin, n] * v_w[c, cin]  -> [128-token chunks, 256]
        for kc in range(KC):
            pv = ps_w.tile([P, 512], F32, tag="ps_w")
            for t in range(2):
                nc.tensor.matmul(pv[:, 0:C], xbf[j][:, t, kc * P:(kc + 1) * P],
                                 wbf["v_wt"][:, t, :], start=(t == 0), stop=(t == 1))
            nc.vector.tensor_copy(vst[j][:, kc, 0:C], pv[:, 0:C])
        nc.vector.memset(vst[j][:, :, 256:257], 1.0)

    def project_qg(i):
        # Qg = (Wk^T Wq / sqrt(C)) @ X_i, query slice only. [c_out, QS]
        for u in range(2):
            for qh in range(2):
                pq = ps_w.tile([P, 512], F32, tag="ps_w")
                for t in range(2):
                    nc.tensor.matmul(
                        pq[:, 0:QH], wbf["m_t"][:, t, u * P:(u + 1) * P],
                        xqbf[i][:, t, qh * QH:(qh + 1) * QH],
                        start=(t == 0), stop=(t == 1))
                nc.vector.tensor_copy(qg[i][:, u, qh * QH:(qh + 1) * QH], pq[:, 0:QH])

    def project_gate(i):
        # gate = sigmoid(Wg X + bg) = 0.5*tanh((Wg X + bg)/2) + 0.5 -- tanh
        # lives in the same ACT table set as exp, so no table switches ever.
        for u in range(2):
            for qh in range(2):
                pg = ps_w.tile([P, 512], F32, tag="ps_w")
                for t in range(2):
                    nc.tensor.matmul(
                        pg[:, 0:QH], wbf["gate_wt"][:, t, u * P:(u + 1) * P],
                        xqbf[i][:, t, qh * QH:(qh + 1) * QH],
                        start=(t == 0), stop=(t == 1))
                sl = gate[i][:, u, qh * QH:(qh + 1) * QH]
                nc.scalar.activation(out=sl, in_=pg[:, 0:QH], func=AF.Tanh,
                                     bias=gb_sb[:, u, :], scale=0.5)
                nc.vector.tensor_scalar(out=sl, in0=sl, scalar1=0.5, scalar2=0.5,
                                        op0=ALU.mult, op1=ALU.add)

    def attention_pair(i, j, ctx_sb):
        # scores^T = X_j^T @ Qg_i, per 128-key chunk; exp -> P^T (bf16);
        # ctx_aug = P^T.T @ [V^T | 1]; normalize+accumulate into ctx_sb.
        pt = pt_pool.tile([P, KC, QS], BF16, tag="pt")
        for kc in range(KC):
            ps = ps_s.tile([P, 2, 512], F32, tag="ps_s")
            for t in range(2):
                for qh in range(2):
                    nc.tensor.matmul(
                        ps[:, qh, 0:QH], xbf[j][:, t, kc * P:(kc + 1) * P],
                        qg[i][:, t, qh * QH:(qh + 1) * QH],
                        start=(t == 0), stop=(t == 1), skip_group_check=True)
            nc.scalar.activation(
                out=pt[:, kc, :].rearrange("p (a b) -> p a b", a=2),
                in_=ps[:, :, 0:QH], func=AF.Exp)
        for mq in range(MQ):
            mm = min(P, QS - mq * P)
            pc = ps_c.tile([P, 512], F32, tag="ps_c")
            for kc in range(KC):
                nc.tensor.matmul(pc[0:mm, 0:257],
                                 pt[:, kc, mq * P:mq * P + mm],
                                 vst[j][:, kc, :],
                                 start=(kc == 0), stop=(kc == KC - 1))
            rp = rp_pool.tile([P, 1], F32, tag="rp")
            nc.vector.reciprocal(rp[0:mm, :], pc[0:mm, 256:257])
            if j == 0:
                nc.vector.tensor_scalar_mul(ctx_sb[0:mm, mq, :], pc[0:mm, 0:C],
                                            rp[0:mm, :])
            else:
                nc.vector.scalar_tensor_tensor(
                    out=ctx_sb[0:mm, mq, :], in0=pc[0:mm, 0:C], scalar=rp[0:mm, :],
                    in1=ctx_sb[0:mm, mq, :], op0=ALU.mult, op1=ALU.add)

    def finish_modality(i, ctx_sb):
        # transpose ctx to [C, QS], add 3*v_b; gate-blend; output conv; DMA out.
        ctx_t = ctxt_pool.tile([P, 2, QS], F32, tag="ctxt")
        for u in range(2):
            for mq in range(MQ):
                mm = min(P, QS - mq * P)
                pt_ps = ps_w.tile([P, 512], F32, tag="ps_w")
                nc.tensor.transpose(pt_ps[:, 0:mm],
                                    ctx_sb[0:mm, mq, u * P:(u + 1) * P],
                                    ident[0:mm, 0:mm])
                nc.vector.tensor_scalar_add(ctx_t[:, u, mq * P:mq * P + mm],
                                            pt_ps[:, 0:mm], vb3_sb[:, u, :])
        fus = fus_pool.tile([P, 2, QS], BF16, tag="fus")
        for u in range(2):
            diff = tmpq_pool.tile([P, QS], F32, tag="tmpq")
            nc.vector.tensor_sub(diff[:, :], ctx_t[:, u, :], xqbf[i][:, u, :])
            nc.vector.tensor_mul(diff[:, :], diff[:, :], gate[i][:, u, :])
            nc.vector.tensor_add(fus[:, u, :], diff[:, :], xqbf[i][:, u, :])
        for u in range(2):
            for qh in range(2):
                po = ps_w.tile([P, 512], F32, tag="ps_w")
                for t in range(2):
                    nc.tensor.matmul(
                        po[:, 0:QH], wbf["out_wt"][:, t, u * P:(u + 1) * P],
                        fus[:, t, qh * QH:(qh + 1) * QH],
                        start=(t == 0), stop=(t == 1))
                osb = osb_pool.tile([P, QH], F32, tag="osb")
                nc.scalar.activation(out=osb[:, :], in_=po[:, 0:QH],
                                     func=AF.Identity, bias=outb_sb[:, u, :],
                                     scale=1.0)
                nc.sync.dma_start(
                    out=io["out"][i, u * P:(u + 1) * P, qh * QH:(qh + 1) * QH],
                    in_=osb[:, :])

    # ---- schedule -------------------------------------------------------
    # Prime the exp table set (contains Identity and Tanh too) off the
    # critical path; the whole kernel then needs exactly one table load.
    nc.scalar.activation(out=gb_sb[0:1, 0, :], in_=gb_sb[0:1, 0, :],
                         func=AF.Exp, bias=0.0, scale=0.0)
    nc.vector.memset(gb_sb[0:1, 0, :], 0.0)
    load_xq(0)
    load_weight("m_t")
    load_weight("gate_wt")
    load_xq(1)
    load_xq(2)
    load_weight("v_wt")
    load_weight("out_wt")
    load_consts()
    for i in range(3):
        project_gate(i)
        project_qg(i)
    load_x(0)
    ident = persist.tile([P, P], F32, tag="ident")
    make_identity(nc, ident)

    project_v(0)
    ctx_sbs = {}
    for i in range(3):
        ctx_sbs[i] = ctx_pool.tile([P, MQ, C], F32, tag="ctxp", name=f"ctx{i}")

    # modality-major pair order; V_j for j>0 is produced while attention on
    # earlier pairs keeps the PE busy.
    attention_pair(0, 0, ctx_sbs[0])
    load_x(1)
    project_v(1)
    attention_pair(0, 1, ctx_sbs[0])
    load_x(2)
    project_v(2)
    attention_pair(0, 2, ctx_sbs[0])
    finish_modality(0, ctx_sbs[0])
    for i in (1, 2):
        for j in range(3):
            attention_pair(i, j, ctx_sbs[i])
        finish_modality(i, ctx_sbs[i])


def _build():
    nc = bacc.Bacc("TRN2", target_bir_lowering=False, debug=False,
                   num_devices=NCORES)
    io = {}
    for j in range(3):
        io[f"x{j}"] = nc.declare_dram_parameter(f"x{j}", [C, N], F32, isOutput=False)
        io[f"xq{j}"] = nc.declare_dram_parameter(f"xq{j}", [C, QS], F32, isOutput=False)
    for wn in ["m_t", "v_wt", "gate_wt", "out_wt"]:
        io[wn] = nc.declare_dram_parameter(wn, [C, C], BF16, isOutput=False)
    for vn in ["pos", "gate_b", "v_b3", "out_b"]:
        io[vn] = nc.declare_dram_parameter(vn, [C], F32, isOutput=False)
    io["out"] = nc.declare_dram_parameter("out", [3, C, QS], F32, isOutput=True)

    with tile.TileContext(nc) as tc:
        with ExitStack() as ctx:
            _emit(ctx, tc, io)
    nc.compile()
    return nc


_CACHED_NC = None


def _get_nc():
    global _CACHED_NC
    if _CACHED_NC is None:
        _CACHED_NC = _build()
    return _CACHED_NC


def _run(inputs: dict, trace: bool = False, tmpdir: str | None = None):
    f32 = np.float32
    bf16 = ml_dtypes.bfloat16
    feats = [np.ascontiguousarray(inputs[f"feat{j}"], dtype=f32).reshape(B, C, N)
             for j in range(3)]
    q_w = np.asarray(inputs["q_w"], f32)
    k_w = np.asarray(inputs["k_w"], f32)
    for bn in ("q_b", "k_b"):
        if not np.all(np.asarray(inputs[bn]) == 0):
            raise NotImplementedError(f"{bn} != 0 unsupported (spec fill=zeros)")
    scale = np.float32(C ** -0.5)
    m_t = np.ascontiguousarray(((q_w.T @ k_w) * scale).astype(bf16))
    v_wt = np.ascontiguousarray(np.asarray(inputs["v_w"], f32).T.astype(bf16))
    gate_wt = np.ascontiguousarray(np.asarray(inputs["gate_w"], f32).T.astype(bf16))
    out_wt = np.ascontiguousarray(np.asarray(inputs["out_w"], f32).T.astype(bf16))
    pos = np.ascontiguousarray(np.asarray(inputs["pos_embedding"], f32).reshape(C))
    gate_b = np.ascontiguousarray(0.5 * np.asarray(inputs["gate_b"], f32).reshape(C))
    v_b3 = np.ascontiguousarray(3.0 * np.asarray(inputs["v_b"], f32).reshape(C))
    out_b = np.ascontiguousarray(np.asarray(inputs["out_b"], f32).reshape(C))

    shared = {"m_t": m_t, "v_wt": v_wt, "gate_wt": gate_wt, "out_wt": out_wt,
              "pos": pos, "gate_b": gate_b, "v_b3": v_b3, "out_b": out_b}
    in_maps = []
    for r in range(NCORES):
        b, s = r // NSLICE, r % NSLICE
        im = dict(shared)
        for j in range(3):
            im[f"x{j}"] = np.ascontiguousarray(feats[j][b])
            im[f"xq{j}"] = np.ascontiguousarray(
                feats[j][b][:, s * QS:(s + 1) * QS])
        in_maps.append(im)

    nc = _get_nc()
    res = run_bass_kernel_spmd(nc, in_maps, core_ids=list(range(NCORES)),
                               trace=trace, tmpdir=tmpdir)
    full = np.empty((3, B, C, N), dtype=f32)
    for r in range(NCORES):
        b, s = r // NSLICE, r % NSLICE
        full[:, b, :, s * QS:(s + 1) * QS] = res.results[r]["out"]
    full = full.reshape(3, B, C, H, W)
    return (full[0], full[1], full[2]), res


def kernel(**inputs):
    outs, _ = _run(inputs, trace=bool(os.environ.get("KERNEL_TRACE")))
    return outs



# revision 14
# speedup vs baseline: 1.3114x; 1.3114x over previous
"""Cross-modality attention Trainium2 kernel (8 NeuronCores, SPMD).

Problem: 3 modalities feat0..2 of [B=2, C=256, H=W=48]. For each modality i:
  ctx_i = sum_j softmax(Q_i K_j^T / sqrt(C)) V_j   (9 full NxN attentions, N=2304)
  out_i = Conv1x1(gate_i * ctx_i + (1-gate_i) * feat_i),  gate_i = sigmoid(Conv1x1(feat_i))

Sharding: core r handles batch b = r//4 and query-token slice s = r%4 (576 of the
2304 tokens) for all 3 modalities, flash-attention style with K/V replicated
(computed on-device from the full per-batch features).

Device algorithm per core (attention matmuls in fp8 e4m3 DoubleRow mode, which
contracts K=256 per instruction at 2x bf16 throughput; fp32 PSUM accumulation):
- Q/K projections are folded into one matrix on the host:
  scores^T = X_j^T (Wk^T Wq / sqrt(C)) X_i = X_j^T Qg_i, so only one projection
  (Qg) per modality and the K projection/storage disappears entirely.
- pos_embedding on the key side adds a per-query constant to scores, which
  softmax cancels -> keys use RAW X_j (host pre-cast to fp8). Its effect on V
  folds into the V bias: vb3 = 3*(v_b + v_w @ pos).
- Scores are computed TRANSPOSED ([key, query] layout) directly by matmul, so no
  transpose of the probability matrix is ever needed. exp() without
  max-subtraction (scores are bounded, |s| < ~1.5 for this distribution); the
  exp is scaled by 1/SQ (Qg is pre-scaled by SQ=64 to keep fp8 out of the
  subnormal range) and biased by log(1/4) for fp8 headroom (cancels in the
  softmax normalization).
- V^T is computed in [token, channel] layout (fp8, pre-scaled by SV=32) with an
  SV-column appended; the attention row-sum (softmax denominator, also scaled
  by SV) falls out of the same matmul that computes P @ V, and normalization is
  a per-partition scalar multiply fused with the cross-modality accumulation.
- Non-PE work is spread across engines: exp+tanh on ACT, reciprocal+blend on
  DVE, PSUM->SBUF casts / normalization / bias-adds on Pool (gpsimd).
- Emission is software-pipelined: PV matmuls of pair p and V/output projections
  are deferred and drained between the score chunks of pair p+1, so the ACT
  engine (exp is the critical resource) never starves.
"""

import math
import os
from contextlib import ExitStack

import ml_dtypes
import numpy as np

import concourse.bass as bass
import concourse.tile as tile
from concourse import bacc, mybir
from concourse.bass_utils import run_bass_kernel_spmd
from concourse.masks import make_identity

B, C, H, W = 2, 256, 48, 48
N = H * W            # 2304 tokens
NCORES = 8
NSLICE = 4           # query slices per batch
QS = N // NSLICE     # 576 query tokens per core
KC = N // 128        # 18 key chunks of 128
QH = QS // 2         # 288: scores free-dim half
QT = QS // 3         # 192: scores free-dim third (fused-exp variant)
MQ = (QS + 127) // 128  # 5 query m-chunks for the ctx matmul (last is 64)

SQ = 64.0            # Qg fp8 pre-scale (exp() divides it back out)
SV = 32.0            # V fp8 pre-scale (cancels in softmax normalization)
LOG_SP = math.log(0.25)  # exp() bias: P = exp(s)/4, fp8 overflow headroom

F32 = mybir.dt.float32
BF16 = mybir.dt.bfloat16
FP8 = mybir.dt.float8e4
AF = mybir.ActivationFunctionType
ALU = mybir.AluOpType
DR = mybir.MatmulPerfMode.DoubleRow

EXPV = int(os.environ.get("EXPV", "1"))  # 1: fused 2-chunk exp (192-packed)


def _emit(ctx: ExitStack, tc: tile.TileContext, io: dict):
    nc = tc.nc

    # ---- pools ----------------------------------------------------------
    P = 128
    persist = ctx.enter_context(tc.tile_pool(name="persist", bufs=1))
    pt_pool = ctx.enter_context(tc.tile_pool(name="pt", bufs=3))
    ctx_pool = ctx.enter_context(tc.tile_pool(name="ctxp", bufs=2))
    ctxb_pool = ctx.enter_context(tc.tile_pool(name="ctxb", bufs=2))
    ctxt_pool = ctx.enter_context(tc.tile_pool(name="ctxt", bufs=2))
    fus_pool = ctx.enter_context(tc.tile_pool(name="fus", bufs=2))
    tmpq_pool = ctx.enter_context(tc.tile_pool(name="tmpq", bufs=2))
    rp_pool = ctx.enter_context(tc.tile_pool(name="rp", bufs=6))
    osb_pool = ctx.enter_context(tc.tile_pool(name="osb", bufs=3))
    ps_s = ctx.enter_context(tc.tile_pool(name="ps_s", bufs=2, space="PSUM"))
    ps_c = ctx.enter_context(tc.tile_pool(name="ps_c", bufs=2, space="PSUM"))

    # ---- constants / weights (host pre-casts & pre-scales) --------------
    gb_sb = persist.tile([P, 2, 1], F32, tag="gb")
    vb3_sb = persist.tile([P, 2, 1], F32, tag="vb3")
    outb_sb = persist.tile([P, 2, 1], F32, tag="outb")
    prime_sb = persist.tile([P, 1], F32, tag="prime")
    lsp_sb = persist.tile([P, 1], F32, tag="lsp")

    wbf = {}
    for wn in ["m_t", "gate_wt", "out_wt"]:
        wbf[wn] = persist.tile([P, 2, C], BF16, tag=f"w_{wn}", name=f"w_{wn}")
    v8t = persist.tile([P, 2, C], FP8, tag="v8t", name="v8t")

    def load_weight(wn):
        nc.sync.dma_start(out=wbf[wn][:, :, :],
                          in_=io[wn].rearrange("(t p) d -> p t d", p=P))

    def load_consts():
        for nm, t_ in (("gate_b", gb_sb), ("v_b3", vb3_sb), ("out_b", outb_sb)):
            nc.sync.dma_start(out=t_[:, :, 0],
                              in_=io[nm].rearrange("(t p) -> p t", p=P))

    x8 = [persist.tile([P, 2, N], FP8, tag=f"x8{j}", name=f"x8{j}")
          for j in range(3)]
    xqbf = [persist.tile([P, 2, QS], BF16, tag=f"xqbf{i}", name=f"xqbf{i}")
            for i in range(3)]
    vst = [persist.tile([P, KC, 264], FP8, tag=f"vst{j}", name=f"vst{j}")
           for j in range(3)]
    qg = [persist.tile([P, 2, QS], FP8, tag=f"qg{i}", name=f"qg{i}")
          for i in range(3)]
    gate = [persist.tile([P, 2, QS], BF16, tag=f"gate{i}", name=f"gate{i}")
            for i in range(3)]

    def load_xq(i):
        nc.sync.dma_start(out=xqbf[i][:, :, :],
                          in_=io[f"xq{i}"].rearrange("(t p) n -> p t n", p=P))

    def load_x(j):
        # chunked [128, 2, 1152] fp8 loads so downstream compute starts early
        for xc in range(2):
            nc.gpsimd.dma_start(
                out=x8[j][:, :, xc * (N // 2):(xc + 1) * (N // 2)],
                in_=io[f"x{j}"].rearrange("(t p) n -> p t n", p=P)
                [:, :, xc * (N // 2):(xc + 1) * (N // 2)])

    def project_v(j, kc):
        # V^T[n, c] = sum_cin X[cin, n] * (SV*v_w)[c, cin] (fp8 DoubleRow)
        pv = ps_c.tile([P, 512], F32, tag="ps_c")
        nc.tensor.matmul(pv[:, 0:C], x8[j][:, :, kc * P:(kc + 1) * P],
                         v8t[:, :, :], start=True, stop=True,
                         perf_mode=DR, skip_group_check=True)
        nc.vector.tensor_copy(vst[j][:, kc, 0:C], pv[:, 0:C])

    def project_qg(i):
        # Qg = (SQ * Wk^T Wq / sqrt(C)) @ Xq_i (bf16), cast fp8 on Pool
        for u in range(2):
            for qh in range(2):
                pq = ps_c.tile([P, 512], F32, tag="ps_c")
                for t in range(2):
                    nc.tensor.matmul(
                        pq[:, 0:QH], wbf["m_t"][:, t, u * P:(u + 1) * P],
                        xqbf[i][:, t, qh * QH:(qh + 1) * QH],
                        start=(t == 0), stop=(t == 1))
                nc.vector.tensor_copy(qg[i][:, u, qh * QH:(qh + 1) * QH],
                                      pq[:, 0:QH])

    def project_gate(i):
        # gate = sigmoid(Wg X + bg) = 0.5*tanh((Wg X + bg)/2) + 0.5 -- tanh
        # lives in the same ACT table set as exp, so no table switches ever.
        for u in range(2):
            for qh in range(2):
                pg = ps_c.tile([P, 512], F32, tag="ps_c")
                for t in range(2):
                    nc.tensor.matmul(
                        pg[:, 0:QH], wbf["gate_wt"][:, t, u * P:(u + 1) * P],
                        xqbf[i][:, t, qh * QH:(qh + 1) * QH],
                        start=(t == 0), stop=(t == 1))
                sl = gate[i][:, u, qh * QH:(qh + 1) * QH]
                nc.scalar.activation(out=sl, in_=pg[:, 0:QH], func=AF.Tanh,
                                     bias=gb_sb[:, u, :], scale=0.5)
                nc.gpsimd.tensor_scalar(out=sl, in0=sl, scalar1=0.5, scalar2=0.5,
                                        op0=ALU.mult, op1=ALU.add)

    # ---- deferred-work queue (software pipelining) -----------------------
    deferred = []

    def drain(n):
        for _ in range(min(n, len(deferred))):
            deferred.pop(0)()

    def emit_scores(i, j, pt):
        # scores^T = X_j^T @ Qg_i (fp8 DoubleRow, one instr per key chunk
        # per query sub-block); exp -> P^T (fp8).
        if EXPV:
            for kp in range(KC // 2):
                ps = ps_s.tile([P, 3, 512], F32, tag="ps_s")
                for h in range(6):
                    kc = 2 * kp + h // 3
                    qh = h % 3
                    bank, half = divmod(h, 2)
                    nc.tensor.matmul(
                        ps[:, bank, half * QT:(half + 1) * QT],
                        x8[j][:, :, kc * P:(kc + 1) * P],
                        qg[i][:, :, qh * QT:(qh + 1) * QT],
                        start=(half == 0), stop=True,
                        perf_mode=DR, skip_group_check=True)
                nc.scalar.activation(
                    out=pt[:, 2 * kp:2 * kp + 2, :].rearrange(
                        "p a b -> p (a b)"),
                    in_=ps[:, :, 0:2 * QT], func=AF.Exp,
                    bias=lsp_sb[:, :], scale=1.0 / SQ)
                drain(2)
        else:
            for kc in range(KC):
                ps = ps_s.tile([P, 2, 512], F32, tag="ps_s")
                for qh in range(2):
                    nc.tensor.matmul(
                        ps[:, qh, 0:QH], x8[j][:, :, kc * P:(kc + 1) * P],
                        qg[i][:, :, qh * QH:(qh + 1) * QH],
                        start=True, stop=True,
                        perf_mode=DR, skip_group_check=True)
                nc.scalar.activation(
                    out=pt[:, kc, :].rearrange("p (a b) -> p a b", a=2),
                    in_=ps[:, :, 0:QH], func=AF.Exp,
                    bias=lsp_sb[:, :], scale=1.0 / SQ)
                drain(1)

    def make_pv(i, j, pt, ctx_sb, ctx_bf):
        # ctx_aug = P^T.T @ [V^T | SV] (fp8 DoubleRow); normalize+accumulate
        # on Pool; the j==2 pass writes the bf16 copy used by the transpose.
        def pv_one(mq):
            def fn():
                mm = min(P, QS - mq * P)
                pc = ps_c.tile([P, 512], F32, tag="ps_c")
                for kp in range(KC // 2):
                    nc.tensor.matmul(pc[0:mm, 0:257],
                                     pt[:, 2 * kp:2 * kp + 2, mq * P:mq * P + mm],
                                     vst[j][:, 2 * kp:2 * kp + 2, 0:257],
                                     start=(kp == 0), stop=(kp == KC // 2 - 1),
                                     perf_mode=DR)
                rp = rp_pool.tile([P, 1], F32, tag="rp")
                nc.vector.reciprocal(rp[0:mm, :], pc[0:mm, 256:257])
                if j == 0:
                    nc.vector.tensor_scalar_mul(ctx_sb[0:mm, mq, :],
                                                pc[0:mm, 0:C], rp[0:mm, :])
                elif j == 1:
                    nc.vector.scalar_tensor_tensor(
                        out=ctx_sb[0:mm, mq, :], in0=pc[0:mm, 0:C],
                        scalar=rp[0:mm, :], in1=ctx_sb[0:mm, mq, :],
                        op0=ALU.mult, op1=ALU.add)
                else:
                    nc.vector.scalar_tensor_tensor(
                        out=ctx_bf[0:mm, mq, :], in0=pc[0:mm, 0:C],
                        scalar=rp[0:mm, :], in1=ctx_sb[0:mm, mq, :],
                        op0=ALU.mult, op1=ALU.add)
            return fn
        return [pv_one(mq) for mq in range(MQ)]

    def make_finish(i, ctx_bf):
        # transpose ctx to [C, QS] (bf16), add vb3; gate-blend; output conv.
        ctx_t = ctxt_pool.tile([P, 2, QS], BF16, tag="ctxt")
        fus = fus_pool.tile([P, 2, QS], BF16, tag="fus")
        items = []

        def tr_one(u, mq):
            def fn():
                mm = min(P, QS - mq * P)
                pt_ps = ps_c.tile([P, 256], BF16, tag="ps_c")
                nc.tensor.transpose(pt_ps[:, 0:mm],
                                    ctx_bf[0:mm, mq, u * P:(u + 1) * P],
                                    ident[0:mm, 0:mm])
                nc.vector.tensor_scalar_add(ctx_t[:, u, mq * P:mq * P + mm],
                                            pt_ps[:, 0:mm], vb3_sb[:, u, :])
            return fn

        def blend(u):
            def fn():
                diff = tmpq_pool.tile([P, QS], BF16, tag="tmpq")
                nc.gpsimd.tensor_sub(diff[:, :], ctx_t[:, u, :], xqbf[i][:, u, :])
                nc.gpsimd.tensor_mul(diff[:, :], diff[:, :], gate[i][:, u, :])
                nc.gpsimd.tensor_add(fus[:, u, :], diff[:, :], xqbf[i][:, u, :])
            return fn

        def out_one(u, qh):
            def fn():
                po = ps_c.tile([P, 512], F32, tag="ps_c")
                for t in range(2):
                    nc.tensor.matmul(
                        po[:, 0:QH], wbf["out_wt"][:, t, u * P:(u + 1) * P],
                        fus[:, t, qh * QH:(qh + 1) * QH],
                        start=(t == 0), stop=(t == 1))
                osb = osb_pool.tile([P, QH], F32, tag="osb")
                nc.vector.tensor_scalar_add(osb[:, :], po[:, 0:QH],
                                            outb_sb[:, u, :])
                nc.sync.dma_start(
                    out=io["out"][i, u * P:(u + 1) * P, qh * QH:(qh + 1) * QH],
                    in_=osb[:, :])
            return fn

        for u in range(2):
            for mq in range(MQ):
                items.append(tr_one(u, mq))
        items.append(blend(0))
        items.append(blend(1))
        for u in range(2):
            for qh in range(2):
                items.append(out_one(u, qh))
        return items

    # ---- schedule -------------------------------------------------------
    # Prime the exp table set (contains Identity and Tanh too) off the
    # critical path; the whole kernel then needs exactly one table load.
    nc.vector.memset(lsp_sb[:, :], LOG_SP)
    nc.scalar.activation(out=prime_sb[0:1, :], in_=prime_sb[0:1, :],
                         func=AF.Exp, bias=0.0, scale=0.0)
    load_weight("m_t")
    load_xq(0)
    load_xq(1)
    load_xq(2)
    load_weight("gate_wt")
    load_consts()
    load_x(0)
    load_weight("out_wt")
    nc.sync.dma_start(out=v8t[:, :, :],
                      in_=io["v_wt8"].rearrange("(t p) d -> p t d", p=P))
    load_x(1)
    load_x(2)
    ident = persist.tile([P, P], BF16, tag="ident")
    make_identity(nc, ident)
    for j in range(3):
        nc.gpsimd.memset(vst[j][:, :, 256:257], SV)

    for i in range(3):
        project_qg(i)
        project_gate(i)
    for kc in range(KC):
        project_v(0, kc)

    pairs = [(i, j) for i in range(3) for j in range(3)]
    ctx_sbs, ctx_bfs = {}, {}
    for i in range(3):
        ctx_sbs[i] = ctx_pool.tile([P, MQ, C], F32, tag="ctxp", name=f"ctx{i}")
        ctx_bfs[i] = ctxb_pool.tile([P, MQ, C], BF16, tag="ctxb", name=f"ctxb{i}")

    for p, (i, j) in enumerate(pairs):
        pt = pt_pool.tile([P, KC, QS], FP8, tag="pt")
        emit_scores(i, j, pt)
        # V_j for j>0 is produced while attention on earlier pairs keeps
        # the PE busy (deferred into the next pair's score stream).
        if p == 0:
            deferred.extend(lambda j_=1, kc_=kc: project_v(j_, kc_)
                            for kc in range(KC))
        elif p == 1:
            deferred.extend(lambda j_=2, kc_=kc: project_v(j_, kc_)
                            for kc in range(KC))
        deferred.extend(make_pv(i, j, pt, ctx_sbs[i], ctx_bfs[i]))
        if j == 2:
            deferred.extend(make_finish(i, ctx_bfs[i]))
    drain(len(deferred))


def _build():
    nc = bacc.Bacc("TRN2", target_bir_lowering=False, debug=False,
                   num_devices=NCORES)
    io = {}
    for j in range(3):
        io[f"x{j}"] = nc.declare_dram_parameter(f"x{j}", [C, N], FP8,
                                                isOutput=False)
        io[f"xq{j}"] = nc.declare_dram_parameter(f"xq{j}", [C, QS], BF16,
                                                 isOutput=False)
    for wn in ["m_t", "gate_wt", "out_wt"]:
        io[wn] = nc.declare_dram_parameter(wn, [C, C], BF16, isOutput=False)
    io["v_wt8"] = nc.declare_dram_parameter("v_wt8", [C, C], FP8, isOutput=False)
    for vn in ["gate_b", "v_b3", "out_b"]:
        io[vn] = nc.declare_dram_parameter(vn, [C], F32, isOutput=False)
    io["out"] = nc.declare_dram_parameter("out", [3, C, QS], F32, isOutput=True)

    with tile.TileContext(nc) as tc:
        with ExitStack() as ctx:
            _emit(ctx, tc, io)
    nc.compile()
    return nc


_CACHED_NC = None


def _get_nc():
    global _CACHED_NC
    if _CACHED_NC is None:
        _CACHED_NC = _build()
    return _CACHED_NC


def _run(inputs: dict, trace: bool = False, tmpdir: str | None = None):
    f32 = np.float32
    bf16 = ml_dtypes.bfloat16
    fp8 = ml_dtypes.float8_e4m3
    feats = [np.ascontiguousarray(inputs[f"feat{j}"], dtype=f32).reshape(B, C, N)
             for j in range(3)]
    q_w = np.asarray(inputs["q_w"], f32)
    k_w = np.asarray(inputs["k_w"], f32)
    v_w = np.asarray(inputs["v_w"], f32)
    for bn in ("q_b", "k_b"):
        if not np.all(np.asarray(inputs[bn]) == 0):
            raise NotImplementedError(f"{bn} != 0 unsupported (spec fill=zeros)")
    scale = np.float32(C ** -0.5)
    pos = np.asarray(inputs["pos_embedding"], f32).reshape(C, 1)
    m_t = np.ascontiguousarray(((q_w.T @ k_w) * (scale * SQ)).astype(bf16))
    v_wt8 = np.ascontiguousarray((v_w.T * SV).astype(fp8))
    gate_wt = np.ascontiguousarray(np.asarray(inputs["gate_w"], f32).T.astype(bf16))
    out_wt = np.ascontiguousarray(np.asarray(inputs["out_w"], f32).T.astype(bf16))
    gate_b = np.ascontiguousarray(0.5 * np.asarray(inputs["gate_b"], f32).reshape(C))
    v_b3 = np.ascontiguousarray(
        3.0 * (np.asarray(inputs["v_b"], f32).reshape(C) + (v_w @ pos).ravel()))
    out_b = np.ascontiguousarray(np.asarray(inputs["out_b"], f32).reshape(C))

    shared = {"m_t": m_t, "v_wt8": v_wt8, "gate_wt": gate_wt, "out_wt": out_wt,
              "gate_b": gate_b, "v_b3": v_b3, "out_b": out_b}
    x8 = [np.ascontiguousarray(feats[j].astype(fp8)) for j in range(3)]
    in_maps = []
    for r in range(NCORES):
        b, s = r // NSLICE, r % NSLICE
        im = dict(shared)
        for j in range(3):
            im[f"x{j}"] = x8[j][b]
            im[f"xq{j}"] = np.ascontiguousarray(
                (feats[j][b][:, s * QS:(s + 1) * QS] + pos).astype(bf16))
        in_maps.append(im)

    nc = _get_nc()
    res = run_bass_kernel_spmd(nc, in_maps, core_ids=list(range(NCORES)),
                               trace=trace, tmpdir=tmpdir)
    full = np.empty((3, B, C, N), dtype=f32)
    for r in range(NCORES):
        b, s = r // NSLICE, r % NSLICE
        full[:, b, :, s * QS:(s + 1) * QS] = res.results[r]["out"]
    full = full.reshape(3, B, C, H, W)
    return (full[0], full[1], full[2]), res


def kernel(**inputs):
    outs, _ = _run(inputs, trace=bool(os.environ.get("KERNEL_TRACE")))
    return outs


# revision 20
# speedup vs baseline: 1.4654x; 1.1174x over previous
"""Cross-modality attention Trainium2 kernel (8 NeuronCores, SPMD).

Problem: 3 modalities feat0..2 of [B=2, C=256, H=W=48]. For each modality i:
  ctx_i = sum_j softmax(Q_i K_j^T / sqrt(C)) V_j   (9 full NxN attentions, N=2304)
  out_i = Conv1x1(gate_i * ctx_i + (1-gate_i) * feat_i),  gate_i = sigmoid(Conv1x1(feat_i))

Sharding: core r handles batch b = r//4 and query-token slice s = r%4 (576 of the
2304 tokens) for all 3 modalities, flash-attention style with K/V replicated
(computed on-device from the full per-batch features).

Device algorithm per core (attention matmuls in fp8 e4m3 DoubleRow mode, which
contracts K=256 per instruction at 2x bf16 throughput; fp32 PSUM accumulation):
- Q/K projections are folded into one matrix on the host:
  scores^T = X_j^T (Wk^T Wq / sqrt(C)) X_i = X_j^T Qg_i, so only one projection
  (Qg) per modality and the K projection/storage disappears entirely.
- pos_embedding on the key side adds a per-query constant to scores, which
  softmax cancels -> keys use RAW X_j (host pre-cast to fp8). Its effect on V
  folds into the V bias: vb3 = 3*(v_b + v_w @ pos).
- Scores are computed TRANSPOSED ([key, query] layout) directly by matmul, so no
  transpose of the probability matrix is ever needed. exp() without
  max-subtraction (scores are bounded, |s| < ~1.5 for this distribution); the
  exp is scaled by 1/SQ (Qg is pre-scaled by SQ=64 to keep fp8 out of the
  subnormal range) and biased by log(1/4) for fp8 headroom (cancels in the
  softmax normalization).
- V^T is computed in [token, channel] layout (fp8, pre-scaled by SV=32) with an
  SV-column appended; the attention row-sum (softmax denominator, also scaled
  by SV) falls out of the same matmul that computes P @ V, and normalization is
  a per-partition scalar multiply fused with the cross-modality accumulation.
- Non-PE work is spread across engines: exp+tanh on ACT, reciprocal+blend on
  DVE, PSUM->SBUF casts / normalization / bias-adds on Pool (gpsimd).
- Emission is software-pipelined: PV matmuls of pair p and V/output projections
  are deferred and drained between the score chunks of pair p+1, so the ACT
  engine (exp is the critical resource) never starves.
"""

import math
import os
from contextlib import ExitStack

import ml_dtypes
import numpy as np

import concourse.bass as bass
import concourse.tile as tile
from concourse import bacc, mybir
from concourse.bass_utils import run_bass_kernel_spmd
from concourse.masks import make_identity

B, C, H, W = 2, 256, 48, 48
N = H * W            # 2304 tokens
NCORES = 8
NSLICE = 4           # query slices per batch
QS = N // NSLICE     # 576 query tokens per core
KC = N // 128        # 18 key chunks of 128
QH = QS // 2         # 288: scores free-dim half
QT = QS // 3         # 192: scores free-dim third (fused-exp variant)
MQ = (QS + 127) // 128  # 5 query m-chunks for the ctx matmul (last is 64)

SQ = 64.0            # Qg fp8 pre-scale (exp() divides it back out)
SV = 32.0            # V fp8 pre-scale (cancels in softmax normalization)
LOG_SP = math.log(0.25)  # exp() bias: P = exp(s)/4, fp8 overflow headroom

F32 = mybir.dt.float32
BF16 = mybir.dt.bfloat16
FP8 = mybir.dt.float8e4
AF = mybir.ActivationFunctionType
ALU = mybir.AluOpType
DR = mybir.MatmulPerfMode.DoubleRow

EXPV = int(os.environ.get("EXPV", "1"))  # 1: fused 2-chunk exp (192-packed)


def _emit(ctx: ExitStack, tc: tile.TileContext, io: dict):
    nc = tc.nc

    # ---- pools ----------------------------------------------------------
    P = 128
    persist = ctx.enter_context(tc.tile_pool(name="persist", bufs=1))
    pt_pool = ctx.enter_context(tc.tile_pool(name="pt", bufs=3))
    ctx_pool = ctx.enter_context(tc.tile_pool(name="ctxp", bufs=2))
    ctxb_pool = ctx.enter_context(tc.tile_pool(name="ctxb", bufs=2))
    ctxt_pool = ctx.enter_context(tc.tile_pool(name="ctxt", bufs=2))
    fus_pool = ctx.enter_context(tc.tile_pool(name="fus", bufs=2))
    tmpq_pool = ctx.enter_context(tc.tile_pool(name="tmpq", bufs=2))
    rp_pool = ctx.enter_context(tc.tile_pool(name="rp", bufs=6))
    osb_pool = ctx.enter_context(tc.tile_pool(name="osb", bufs=3))
    ps_s = ctx.enter_context(tc.tile_pool(name="ps_s", bufs=2 if EXPV else 3,
                                          space="PSUM"))
    ps_c = ctx.enter_context(tc.tile_pool(name="ps_c", bufs=2, space="PSUM"))

    # ---- constants / weights (host pre-casts & pre-scales) --------------
    gb_sb = persist.tile([P, 2, 1], F32, tag="gb")
    vb3_sb = persist.tile([P, 2, 1], F32, tag="vb3")
    outb_sb = persist.tile([P, 2, 1], F32, tag="outb")
    prime_sb = persist.tile([P, 1], F32, tag="prime")
    lsp_sb = persist.tile([P, 1], F32, tag="lsp")

    wbf = {}
    for wn in ["m_t", "gate_wt", "out_wt"]:
        wbf[wn] = persist.tile([P, 2, C], BF16, tag=f"w_{wn}", name=f"w_{wn}")
    v8t = persist.tile([P, 2, C], FP8, tag="v8t", name="v8t")

    def load_weight(wn):
        nc.sync.dma_start(out=wbf[wn][:, :, :],
                          in_=io[wn].rearrange("(t p) d -> p t d", p=P))

    def load_consts():
        for nm, t_ in (("gate_b", gb_sb), ("v_b3", vb3_sb), ("out_b", outb_sb)):
            nc.sync.dma_start(out=t_[:, :, 0],
                              in_=io[nm].rearrange("(t p) -> p t", p=P))

    x8 = [persist.tile([P, 2, N], FP8, tag=f"x8{j}", name=f"x8{j}")
          for j in range(3)]
    xqbf = [persist.tile([P, 2, QS], BF16, tag=f"xqbf{i}", name=f"xqbf{i}")
            for i in range(3)]
    vst = [persist.tile([P, KC, 264], FP8, tag=f"vst{j}", name=f"vst{j}")
           for j in range(3)]
    qg = [persist.tile([P, 2, QS], FP8, tag=f"qg{i}", name=f"qg{i}")
          for i in range(3)]
    gate = [persist.tile([P, 2, QS], BF16, tag=f"gate{i}", name=f"gate{i}")
            for i in range(3)]

    def load_xq(i):
        nc.sync.dma_start(out=xqbf[i][:, :, :],
                          in_=io[f"xq{i}"].rearrange("(t p) n -> p t n", p=P))

    def load_x(j):
        # chunked [128, 2, 1152] fp8 loads so downstream compute starts early
        for xc in range(2):
            nc.gpsimd.dma_start(
                out=x8[j][:, :, xc * (N // 2):(xc + 1) * (N // 2)],
                in_=io[f"x{j}"].rearrange("(t p) n -> p t n", p=P)
                [:, :, xc * (N // 2):(xc + 1) * (N // 2)])

    def project_v(j, kc):
        # V^T[n, c] = sum_cin X[cin, n] * (SV*v_w)[c, cin] (fp8 DoubleRow)
        pv = ps_c.tile([P, 512], F32, tag="ps_c")
        nc.tensor.matmul(pv[:, 0:C], x8[j][:, :, kc * P:(kc + 1) * P],
                         v8t[:, :, :], start=True, stop=True,
                         perf_mode=DR, skip_group_check=True)
        nc.vector.tensor_copy(vst[j][:, kc, 0:C], pv[:, 0:C])

    def project_qg(i):
        # Qg = (SQ * Wk^T Wq / sqrt(C)) @ Xq_i (bf16), cast fp8 on Pool
        for u in range(2):
            for qh in range(2):
                pq = ps_c.tile([P, 512], F32, tag="ps_c")
                for t in range(2):
                    nc.tensor.matmul(
                        pq[:, 0:QH], wbf["m_t"][:, t, u * P:(u + 1) * P],
                        xqbf[i][:, t, qh * QH:(qh + 1) * QH],
                        start=(t == 0), stop=(t == 1))
                nc.vector.tensor_copy(qg[i][:, u, qh * QH:(qh + 1) * QH],
                                      pq[:, 0:QH])

    def project_gate(i):
        # gate = sigmoid(Wg X + bg) = 0.5*tanh((Wg X + bg)/2) + 0.5 -- tanh
        # lives in the same ACT table set as exp, so no table switches ever.
        for u in range(2):
            for qh in range(2):
                pg = ps_c.tile([P, 512], F32, tag="ps_c")
                for t in range(2):
                    nc.tensor.matmul(
                        pg[:, 0:QH], wbf["gate_wt"][:, t, u * P:(u + 1) * P],
                        xqbf[i][:, t, qh * QH:(qh + 1) * QH],
                        start=(t == 0), stop=(t == 1))
                sl = gate[i][:, u, qh * QH:(qh + 1) * QH]
                nc.scalar.activation(out=sl, in_=pg[:, 0:QH], func=AF.Tanh,
                                     bias=gb_sb[:, u, :], scale=0.5)
                nc.gpsimd.tensor_scalar(out=sl, in0=sl, scalar1=0.5, scalar2=0.5,
                                        op0=ALU.mult, op1=ALU.add)

    # ---- deferred-work queue (software pipelining) -----------------------
    deferred = []

    def drain(n):
        for _ in range(min(n, len(deferred))):
            deferred.pop(0)()

    def emit_scores(i, j, pt):
        # scores^T = X_j^T @ Qg_i (fp8 DoubleRow, one instr per key chunk
        # per query sub-block); exp -> P^T (fp8).
        if EXPV:
            for kp in range(KC // 2):
                ps = ps_s.tile([P, 3, 512], F32, tag="ps_s")
                for h in range(6):
                    kc = 2 * kp + h // 3
                    qh = h % 3
                    bank, half = divmod(h, 2)
                    # each matmul is its own group; hw start_tensor_calc
                    # zero-fills only the bytes it writes, so two groups can
                    # share a bank at different offsets
                    nc.tensor.matmul(
                        ps[:, bank, half * QT:(half + 1) * QT],
                        x8[j][:, :, kc * P:(kc + 1) * P],
                        qg[i][:, :, qh * QT:(qh + 1) * QT],
                        start=True, stop=True,
                        perf_mode=DR, skip_group_check=True)
                nc.scalar.activation(
                    out=pt[:, 2 * kp:2 * kp + 2, :].rearrange(
                        "p a b -> p (a b)"),
                    in_=ps[:, :, 0:2 * QT], func=AF.Exp,
                    bias=lsp_sb[:, :], scale=1.0 / SQ)
                drain(2)
        else:
            for kc in range(KC):
                ps = ps_s.tile([P, 2, 512], F32, tag="ps_s")
                for qh in range(2):
                    nc.tensor.matmul(
                        ps[:, qh, 0:QH], x8[j][:, :, kc * P:(kc + 1) * P],
                        qg[i][:, :, qh * QH:(qh + 1) * QH],
                        start=True, stop=True,
                        perf_mode=DR, skip_group_check=True)
                nc.scalar.activation(
                    out=pt[:, kc, :].rearrange("p (a b) -> p a b", a=2),
                    in_=ps[:, :, 0:QH], func=AF.Exp,
                    bias=lsp_sb[:, :], scale=1.0 / SQ)
                drain(1)

    def make_pv(i, j, pt, ctx_sb, ctx_bf, finish=False):
        # ctx_aug = P^T.T @ [V^T | SV] (fp8 DoubleRow); normalize+accumulate
        # on DVE; the j==2 pass writes the bf16 copy used by the transpose
        # and interleaves the per-mq transposes/blend/out-conv of the
        # modality so the finish chain overlaps the next pair's scores.
        if finish:
            ctx_t = ctxt_pool.tile([P, 2, QS], BF16, tag="ctxt")
            fus = fus_pool.tile([P, 2, QS], BF16, tag="fus")

        def pv_one(mq):
            def fn():
                mm = min(P, QS - mq * P)
                pc = ps_c.tile([P, 512], F32, tag="ps_c")
                for kp in range(KC // 2):
                    nc.tensor.matmul(pc[0:mm, 0:257],
                                     pt[:, 2 * kp:2 * kp + 2, mq * P:mq * P + mm],
                                     vst[j][:, 2 * kp:2 * kp + 2, 0:257],
                                     start=(kp == 0), stop=(kp == KC // 2 - 1),
                                     perf_mode=DR)
                rp = rp_pool.tile([P, 1], F32, tag="rp")
                nc.vector.reciprocal(rp[0:mm, :], pc[0:mm, 256:257])
                if j == 0:
                    nc.vector.tensor_scalar_mul(ctx_sb[0:mm, mq, :],
                                                pc[0:mm, 0:C], rp[0:mm, :])
                elif j == 1:
                    nc.vector.scalar_tensor_tensor(
                        out=ctx_sb[0:mm, mq, :], in0=pc[0:mm, 0:C],
                        scalar=rp[0:mm, :], in1=ctx_sb[0:mm, mq, :],
                        op0=ALU.mult, op1=ALU.add)
                else:
                    nc.vector.scalar_tensor_tensor(
                        out=ctx_bf[0:mm, mq, :], in0=pc[0:mm, 0:C],
                        scalar=rp[0:mm, :], in1=ctx_sb[0:mm, mq, :],
                        op0=ALU.mult, op1=ALU.add)
            return fn

        def tr_one(u, mq):
            def fn():
                mm = min(P, QS - mq * P)
                pt_ps = ps_c.tile([P, 256], BF16, tag="ps_c")
                nc.tensor.transpose(pt_ps[:, 0:mm],
                                    ctx_bf[0:mm, mq, u * P:(u + 1) * P],
                                    ident[0:mm, 0:mm])
                nc.vector.tensor_scalar_add(ctx_t[:, u, mq * P:mq * P + mm],
                                            pt_ps[:, 0:mm], vb3_sb[:, u, :])
            return fn

        def blend(u):
            def fn():
                diff = tmpq_pool.tile([P, QS], BF16, tag="tmpq")
                nc.vector.tensor_sub(diff[:, :], ctx_t[:, u, :], xqbf[i][:, u, :])
                nc.vector.tensor_mul(diff[:, :], diff[:, :], gate[i][:, u, :])
                nc.vector.tensor_add(fus[:, u, :], diff[:, :], xqbf[i][:, u, :])
            return fn

        def out_one(u, qh):
            def fn():
                po = ps_c.tile([P, 512], F32, tag="ps_c")
                for t in range(2):
                    nc.tensor.matmul(
                        po[:, 0:QH], wbf["out_wt"][:, t, u * P:(u + 1) * P],
                        fus[:, t, qh * QH:(qh + 1) * QH],
                        start=(t == 0), stop=(t == 1))
                osb = osb_pool.tile([P, QH], F32, tag="osb")
                nc.vector.tensor_scalar_add(osb[:, :], po[:, 0:QH],
                                            outb_sb[:, u, :])
                nc.sync.dma_start(
                    out=io["out"][i, u * P:(u + 1) * P, qh * QH:(qh + 1) * QH],
                    in_=osb[:, :])
            return fn

        items = []
        for mq in range(MQ):
            items.append(pv_one(mq))
            if finish:
                items.append(tr_one(0, mq))
                items.append(tr_one(1, mq))
        if finish:
            items.append(blend(0))
            items.append(blend(1))
            for u in range(2):
                for qh in range(2):
                    items.append(out_one(u, qh))
        return items

    # ---- schedule -------------------------------------------------------
    # Prime the exp table set (contains Identity and Tanh too) off the
    # critical path; the whole kernel then needs exactly one table load.
    nc.vector.memset(lsp_sb[:, :], LOG_SP)
    nc.scalar.activation(out=prime_sb[0:1, :], in_=prime_sb[0:1, :],
                         func=AF.Exp, bias=0.0, scale=0.0)
    load_weight("m_t")
    load_xq(0)
    load_x(0)
    load_xq(1)
    load_xq(2)
    load_weight("gate_wt")
    load_consts()
    nc.sync.dma_start(out=v8t[:, :, :],
                      in_=io["v_wt8"].rearrange("(t p) d -> p t d", p=P))
    load_weight("out_wt")
    load_x(1)
    load_x(2)
    ident = persist.tile([P, P], BF16, tag="ident")
    make_identity(nc, ident)
    for j in range(3):
        nc.gpsimd.memset(vst[j][:, :, 256:257], SV)

    # only qg(0) is needed before the first score matmul; everything else
    # (other projections, V, gates) drains through the deferred queue.
    project_qg(0)

    def vproj_items(j):
        def two(kc):
            def fn():
                project_v(j, kc)
                project_v(j, kc + 1)
            return fn
        return [two(kc) for kc in range(0, KC, 2)]

    pairs = [(i, j) for i in range(3) for j in range(3)]
    ctx_sbs, ctx_bfs = {}, {}
    for i in range(3):
        ctx_sbs[i] = ctx_pool.tile([P, MQ, C], F32, tag="ctxp", name=f"ctx{i}")
        ctx_bfs[i] = ctxb_pool.tile([P, MQ, C], BF16, tag="ctxb", name=f"ctxb{i}")

    for p, (i, j) in enumerate(pairs):
        if p == 0:
            deferred.extend(vproj_items(0))
        elif p == 1:
            deferred.extend(vproj_items(1))
            deferred.append(lambda: project_qg(1))
            deferred.append(lambda: project_qg(2))
        elif p == 2:
            deferred.extend(vproj_items(2))
        if j == 1:
            deferred.append(lambda i_=i: project_gate(i_))
        pt = pt_pool.tile([P, KC, QS], FP8, tag="pt")
        emit_scores(i, j, pt)
        deferred.extend(make_pv(i, j, pt, ctx_sbs[i], ctx_bfs[i],
                                finish=(j == 2)))
    drain(len(deferred))


def _build():
    nc = bacc.Bacc("TRN2", target_bir_lowering=False, debug=False,
                   num_devices=NCORES)
    io = {}
    for j in range(3):
        io[f"x{j}"] = nc.declare_dram_parameter(f"x{j}", [C, N], FP8,
                                                isOutput=False)
        io[f"xq{j}"] = nc.declare_dram_parameter(f"xq{j}", [C, QS], BF16,
                                                 isOutput=False)
    for wn in ["m_t", "gate_wt", "out_wt"]:
        io[wn] = nc.declare_dram_parameter(wn, [C, C], BF16, isOutput=False)
    io["v_wt8"] = nc.declare_dram_parameter("v_wt8", [C, C], FP8, isOutput=False)
    for vn in ["gate_b", "v_b3", "out_b"]:
        io[vn] = nc.declare_dram_parameter(vn, [C], F32, isOutput=False)
    io["out"] = nc.declare_dram_parameter("out", [3, C, QS], F32, isOutput=True)

    with tile.TileContext(nc) as tc:
        with ExitStack() as ctx:
            _emit(ctx, tc, io)
    nc.compile()
    return nc


_CACHED_NC = None


def _get_nc():
    global _CACHED_NC
    if _CACHED_NC is None:
        _CACHED_NC = _build()
    return _CACHED_NC


def _run(inputs: dict, trace: bool = False, tmpdir: str | None = None):
    f32 = np.float32
    bf16 = ml_dtypes.bfloat16
    fp8 = ml_dtypes.float8_e4m3
    feats = [np.ascontiguousarray(inputs[f"feat{j}"], dtype=f32).reshape(B, C, N)
             for j in range(3)]
    q_w = np.asarray(inputs["q_w"], f32)
    k_w = np.asarray(inputs["k_w"], f32)
    v_w = np.asarray(inputs["v_w"], f32)
    for bn in ("q_b", "k_b"):
        if not np.all(np.asarray(inputs[bn]) == 0):
            raise NotImplementedError(f"{bn} != 0 unsupported (spec fill=zeros)")
    scale = np.float32(C ** -0.5)
    pos = np.asarray(inputs["pos_embedding"], f32).reshape(C, 1)
    m_t = np.ascontiguousarray(((q_w.T @ k_w) * (scale * SQ)).astype(bf16))
    v_wt8 = np.ascontiguousarray((v_w.T * SV).astype(fp8))
    gate_wt = np.ascontiguousarray(np.asarray(inputs["gate_w"], f32).T.astype(bf16))
    out_wt = np.ascontiguousarray(np.asarray(inputs["out_w"], f32).T.astype(bf16))
    gate_b = np.ascontiguousarray(0.5 * np.asarray(inputs["gate_b"], f32).reshape(C))
    v_b3 = np.ascontiguousarray(
        3.0 * (np.asarray(inputs["v_b"], f32).reshape(C) + (v_w @ pos).ravel()))
    out_b = np.ascontiguousarray(np.asarray(inputs["out_b"], f32).reshape(C))

    shared = {"m_t": m_t, "v_wt8": v_wt8, "gate_wt": gate_wt, "out_wt": out_wt,
              "gate_b": gate_b, "v_b3": v_b3, "out_b": out_b}
    x8 = [np.ascontiguousarray(feats[j].astype(fp8)) for j in range(3)]
    in_maps = []
    for r in range(NCORES):
        b, s = r // NSLICE, r % NSLICE
        im = dict(shared)
        for j in range(3):
            im[f"x{j}"] = x8[j][b]
            im[f"xq{j}"] = np.ascontiguousarray(
                (feats[j][b][:, s * QS:(s + 1) * QS] + pos).astype(bf16))
        in_maps.append(im)

    nc = _get_nc()
    res = run_bass_kernel_spmd(nc, in_maps, core_ids=list(range(NCORES)),
                               trace=trace, tmpdir=tmpdir)
    full = np.empty((3, B, C, N), dtype=f32)
    for r in range(NCORES):
        b, s = r // NSLICE, r % NSLICE
        full[:, b, :, s * QS:(s + 1) * QS] = res.results[r]["out"]
    full = full.reshape(3, B, C, H, W)
    return (full[0], full[1], full[2]), res


def kernel(**inputs):
    outs, _ = _run(inputs, trace=bool(os.environ.get("KERNEL_TRACE")))
    return outs
